# revision 1
# baseline (speedup 1.0000x reference)
"""Block-diagonal 2x2 equalizer kernel for Trainium2 (8 NeuronCores).

Per point (b, u, s, f) solves the 2x2 system M x = v by Cramer's rule:
    m_ij = h[b, pi[u], i, 0, 2u+j, s, f]   (only 1/4 of h is needed)
    det  = m00*m11 - m01*m10
    x0   = (m11*v0 - m01*v1) / det
    x1   = (m00*v1 - m10*v0) / det
    out[b, u, a, s, f] = x_a

Sharding: data-parallel over batch, 2 batches per core on 8 cores. The host
gathers (precoding_ind) and packs operand planes into contiguous [128, fd]
blocks so every device DMA is a large fully-contiguous transfer.

Device kernel is raw Bass (no TileContext): the neuronxcc walrus used by the
axon/bass2jax path allows only one sync-wait per instruction, so all waits
are standalone wait_ge instructions and every SBUF buffer is written exactly
once (pure dataflow, per-chunk semaphores, no WAR hazards, no tail barrier).

Pipeline (NCH chunks over the u axis):
  sync engine:  per chunk, loads A={m00|m11}, B={m01|m10}, Y={v0|v1}
  DVE:          all 11 tensor ops per chunk (p0, p1, det, q0, q1, r0, q2,
                q3, r1, x0, x1). GPSIMD is intentionally UNUSED: measured
                on HW, concurrent GPSIMD+DVE contend for SBUF ports and
                drop combined throughput below DVE alone (DVE TT 1.09us
                -> 2.9us while GPSIMD runs).
  ACT (scalar): rdet = Reciprocal(det) via direct InstActivation (HW
                spline measured 2.2e-5 max rel err, 1.04us vs 5.75us for
                DVE reciprocal at FD=896); also issues the stores
"""

from contextlib import ExitStack

import numpy as np

import concourse.bass as bass
import concourse.mybir as mybir
from concourse.bass_utils import run_bass_kernel_spmd

# Problem shapes (hardcoded per contract)
B, U, A, NTX, T, S, F = 16, 4, 2, 1, 8, 14, 2048
SF = S * F               # 28672
NCORES = 8
BPC = B // NCORES        # 2 batches per core
NCH = 2                  # pipeline chunks (groups of u)
UPC = U // NCH           # u's per chunk
QW = 448                 # inner width: SF = 64 * 448
ROWS = SF // QW          # 64 rows -> partition p = b*64 + row
FD = UPC * QW            # free elems per component per chunk

# Set by test harness to capture an NTFF profile on the run.
TRACE = False
LAST_RESULTS = None


def _pack(d):
    """[BPC, U, SF] -> [NCH, 128, FD] with p = b*ROWS + sf//QW, f = ul*QW + sf%QW."""
    d = d.reshape(BPC, U, ROWS, QW)
    out = np.empty((NCH, BPC * ROWS, FD), np.float32)
    for k in range(NCH):
        blk = d[:, k * UPC:(k + 1) * UPC]               # [BPC, UPC, ROWS, QW]
        out[k] = blk.transpose(0, 2, 1, 3).reshape(BPC * ROWS, FD)
    return out


def _unpack(t):
    """Inverse of _pack: [NCH, 128, FD] -> [BPC, U, SF]."""
    out = np.empty((BPC, U, ROWS, QW), np.float32)
    for k in range(NCH):
        blk = t[k].reshape(BPC, ROWS, UPC, QW).transpose(0, 2, 1, 3)
        out[:, k * UPC:(k + 1) * UPC] = blk
    return out.reshape(BPC, U, SF)


def _build_nc():
    f32 = mybir.dt.float32
    nc = bass.Bass("TRN2")
    # hA: [m00 | m11], hB: [m01 | m10], yB: [v0 | v1], xout: [x0 | x1]
    hA = nc.dram_tensor("hA", [NCH, 128, 2 * FD], f32, kind="ExternalInput")
    hB = nc.dram_tensor("hB", [NCH, 128, 2 * FD], f32, kind="ExternalInput")
    yB = nc.dram_tensor("yB", [NCH, 128, 2 * FD], f32, kind="ExternalInput")
    xout = nc.dram_tensor("xout", [NCH, 128, 2 * FD], f32, kind="ExternalOutput")

    with ExitStack() as ctx:
        tA = [ctx.enter_context(nc.sbuf_tensor(f"tA{k}", [128, 2 * FD], f32)) for k in range(NCH)]
        tB = [ctx.enter_context(nc.sbuf_tensor(f"tB{k}", [128, 2 * FD], f32)) for k in range(NCH)]
        tY = [ctx.enter_context(nc.sbuf_tensor(f"tY{k}", [128, 2 * FD], f32)) for k in range(NCH)]
        tX = [ctx.enter_context(nc.sbuf_tensor(f"tX{k}", [128, 2 * FD], f32)) for k in range(NCH)]
        tp = [
            {
                n: ctx.enter_context(nc.sbuf_tensor(f"{n}_{k}", [128, FD], f32))
                for n in ("p0", "p1", "q2", "q3", "det", "rdet", "q0", "q1", "r0", "r1")
            }
            for k in range(NCH)
        ]
        semA = [ctx.enter_context(nc.semaphore(f"semA{k}")) for k in range(NCH)]
        semB = [ctx.enter_context(nc.semaphore(f"semB{k}")) for k in range(NCH)]
        semY = [ctx.enter_context(nc.semaphore(f"semY{k}")) for k in range(NCH)]
        semO = [ctx.enter_context(nc.semaphore(f"semO{k}")) for k in range(NCH)]
        dve_sem = ctx.enter_context(nc.semaphore("dve_sem"))
        act_sem = ctx.enter_context(nc.semaphore("act_sem"))

        with nc.Block() as block:

            @block.sync
            def _(sync):
                for k in range(NCH):
                    sync.dma_start(out=tA[k][:], in_=hA[k]).then_inc(semA[k], 16)
                    sync.dma_start(out=tY[k][:], in_=yB[k]).then_inc(semY[k], 16)
                    sync.dma_start(out=tB[k][:], in_=hB[k]).then_inc(semB[k], 16)

            # dve_sem counts: chunk k ops are 11k+1 .. 11k+11
            # order chosen so every consumer is >=2 ops after its producers
            # (a wait_ge on a just-finished DVE op stalls ~1-2us for the
            # producer's pipe DRAIN + sem propagation; with distance the
            # waits are already satisfied): p0 q0 q2 p1 q1 q3 det r0 r1 x0 x1
            @block.vector
            def _(vector):
                for k in range(NCH):
                    a, b, y, x, t = tA[k], tB[k], tY[k], tX[k], tp[k]
                    m00, m11 = a[:, :FD], a[:, FD:]
                    m01, m10 = b[:, :FD], b[:, FD:]
                    v0, v1 = y[:, :FD], y[:, FD:]
                    c = 11 * k
                    vector.wait_ge(semA[k], 16)
                    vector.tensor_mul(t["p0"][:], m00, m11).then_inc(dve_sem, 1)   # c+1
                    vector.wait_ge(semY[k], 16)
                    vector.tensor_mul(t["q0"][:], m11, v0).then_inc(dve_sem, 1)    # c+2
                    vector.tensor_mul(t["q2"][:], m00, v1).then_inc(dve_sem, 1)    # c+3
                    vector.wait_ge(semB[k], 16)
                    vector.tensor_mul(t["p1"][:], m01, m10).then_inc(dve_sem, 1)   # c+4
                    vector.tensor_mul(t["q1"][:], m01, v1).then_inc(dve_sem, 1)    # c+5
                    vector.tensor_mul(t["q3"][:], m10, v0).then_inc(dve_sem, 1)    # c+6
                    vector.wait_ge(dve_sem, c + 4)
                    vector.tensor_sub(t["det"][:], t["p0"][:], t["p1"][:]).then_inc(
                        dve_sem, 1
                    )  # c+7  (ACT recip consumes)
                    vector.wait_ge(dve_sem, c + 5)
                    vector.tensor_sub(t["r0"][:], t["q0"][:], t["q1"][:]).then_inc(
                        dve_sem, 1
                    )  # c+8
                    vector.wait_ge(dve_sem, c + 6)
                    vector.tensor_sub(t["r1"][:], t["q2"][:], t["q3"][:]).then_inc(
                        dve_sem, 1
                    )  # c+9
                    vector.wait_ge(dve_sem, c + 8)
                    vector.wait_ge(act_sem, k + 1)
                    vector.tensor_mul(x[:, :FD], t["r0"][:], t["rdet"][:]).then_inc(
                        dve_sem, 1
                    )  # c+10 (store x0 consumes)
                    vector.wait_ge(dve_sem, c + 9)
                    vector.tensor_mul(x[:, FD:], t["r1"][:], t["rdet"][:]).then_inc(
                        dve_sem, 1
                    )  # c+11 (store x1 consumes)

            @block.scalar
            def _(scalar):
                for k in range(NCH):
                    c = 11 * k
                    scalar.wait_ge(dve_sem, c + 7)
                    scalar.add_instruction(
                        mybir.InstActivation(
                            name=nc.get_next_instruction_name(),
                            func=mybir.ActivationFunctionType.Reciprocal,
                            ins=[
                                scalar.lower_ap(tp[k]["det"][:]),
                                mybir.ImmediateValue(dtype=f32, value=0.0),
                                mybir.ImmediateValue(dtype=f32, value=1.0),
                                mybir.ImmediateValue(dtype=f32, value=0.0),
                            ],
                            outs=[scalar.lower_ap(tp[k]["rdet"][:])],
                        )
                    ).then_inc(act_sem, 1)
                    scalar.wait_ge(dve_sem, c + 10)
                    scalar.dma_start(out=xout[k, :, :FD], in_=tX[k][:, :FD]).then_inc(
                        semO[k], 16
                    )
                    scalar.wait_ge(dve_sem, c + 11)
                    scalar.dma_start(out=xout[k, :, FD:], in_=tX[k][:, FD:]).then_inc(
                        semO[k], 16
                    )
                for k in range(NCH):
                    scalar.wait_ge(semO[k], 32)

    return nc


def make_in_maps(y, h, precoding_ind):
    """Host-side gather + pack. Returns per-core input maps."""
    y = np.asarray(y)
    h = np.asarray(h)
    pi = np.asarray(precoding_ind).astype(np.int64)

    hg = h[:, pi[0]]                                     # [B, U, A, NTX, T, S, F]
    # hsel[b, u, i, j] = hg[b, u, i, 0, 2u+j]  -> components c = i*2+j
    hsel = np.stack(
        [hg[:, u, :, 0, 2 * u:2 * u + 2] for u in range(U)], axis=1
    )                                                    # [B, U, A(i), 2(j), S, F]
    hsel = np.ascontiguousarray(hsel).reshape(B, U, 4, SF).astype(np.float32)
    yr = np.ascontiguousarray(y).reshape(B, U, A, SF).astype(np.float32)

    in_maps = []
    for c in range(NCORES):
        b0 = c * BPC
        hs = hsel[b0:b0 + BPC]                           # [BPC, U, 4, SF]
        ys = yr[b0:b0 + BPC]                             # [BPC, U, A, SF]
        hA = np.concatenate([_pack(hs[:, :, 0]), _pack(hs[:, :, 3])], axis=2)
        hB = np.concatenate([_pack(hs[:, :, 1]), _pack(hs[:, :, 2])], axis=2)
        yB = np.concatenate([_pack(ys[:, :, 0]), _pack(ys[:, :, 1])], axis=2)
        in_maps.append({
            "hA": np.ascontiguousarray(hA),
            "hB": np.ascontiguousarray(hB),
            "yB": np.ascontiguousarray(yB),
        })
    return in_maps


def assemble_output(results):
    """Per-core xout [NCH, 128, 2FD] -> full [B, U, A, S, F]."""
    out = np.empty((B, U, A, S, F), np.float32)
    for c in range(NCORES):
        xo = np.asarray(results[c]["xout"])
        x0 = _unpack(xo[:, :, :FD]).reshape(BPC, U, S, F)
        x1 = _unpack(xo[:, :, FD:]).reshape(BPC, U, S, F)
        out[c * BPC:(c + 1) * BPC, :, 0] = x0
        out[c * BPC:(c + 1) * BPC, :, 1] = x1
    return out


def kernel(y, h, precoding_ind):
    global LAST_RESULTS
    in_maps = make_in_maps(y, h, precoding_ind)
    nc = _build_nc()
    res = run_bass_kernel_spmd(nc, in_maps, list(range(NCORES)), trace=TRACE)
    LAST_RESULTS = res
    return assemble_output(res.results)



# revision 2
# speedup vs baseline: 1.1315x; 1.1315x over previous
"""Block-diagonal 2x2 equalizer kernel for Trainium2 (8 NeuronCores).

Per point (b, u, s, f) solves the 2x2 system M x = v by Cramer's rule:
    m_ij = h[b, pi[u], i, 0, 2u+j, s, f]   (only 1/4 of h is needed)
    det  = m00*m11 - m01*m10
    x0   = (m11*v0 - m01*v1) / det
    x1   = (m00*v1 - m10*v0) / det
    out[b, u, a, s, f] = x_a

Sharding: data-parallel over batch, 2 batches per core on 8 cores. The host
gathers (precoding_ind) and packs operand planes into contiguous [128, FD]
blocks so every device DMA is a large fully-contiguous transfer.

Precision (validated against the f32 reference on the real inputs,
rel err 3.3e-4 vs the 2e-2 gate):
  - h planes stay f32: det = p0 - p1 cancels to ~1e-4 with |p| ~ 9, and the
    max-|x| point of the output IS the most singular block, so det needs
    f32 products of f32 inputs.
  - y, q, r, rdet, x are fp16: the r-path error is linear (no cancellation
    amplification at the near-singular points that dominate the metric),
    which halves their DMA bytes and doubles DVE throughput (2x_1P mode).

Device kernel is raw Bass (no TileContext): the neuronxcc walrus used by the
axon/bass2jax path allows only one sync-wait per instruction, so all waits
are standalone wait_ge instructions and every SBUF buffer is written exactly
once (pure dataflow, per-chunk semaphores, no WAR hazards).

Pipeline (NCH chunks over the u axis), per chunk:
  sync:  gated serial loads h(m00,m11) -> h(m01,m10) -> y(v0,v1); the gating
         keeps the first mul's operands from sharing DMA bandwidth with
         later transfers (concurrent HWDGE DMAs round-robin at packet
         granularity, so ungated loads all finish together - the baseline's
         DVE sat idle until 15.5us of a 44.6us kernel because of this).
  DVE:   mulP0 p0=m00*m11 (f32), mulP1 p1=m01*m10 (f32), subDet (f32),
         mulQA [q0|q3]=[m11|m10]*v0bcast (fp16), mulQB [q2|q1]=[m00|m01]*v1b,
         subR [r0|r1]=[q0|q2]-[q1|q3], final [x0|x1]=[r0|r1]*rdet_bcast.
         Product pairs are packed into single ops via strided 3D APs
         (plane orders chosen so every AP step is non-negative).
  ACT:   cast H f32->fp16 (for the q muls), Reciprocal(det)->rdet fp16 via
         raw InstActivation (HW spline, 2.2e-5 max rel err), issues stores.
"""

from contextlib import ExitStack

import numpy as np

import concourse.bass as bass
import concourse.mybir as mybir
from concourse.bass_utils import run_bass_kernel_spmd

# Problem shapes (hardcoded per contract)
B, U, A, NTX, T, S, F = 16, 4, 2, 1, 8, 14, 2048
SF = S * F               # 28672
NCORES = 8
BPC = B // NCORES        # 2 batches per core
NCH = 2                  # pipeline chunks (groups of u)
UPC = U // NCH           # u's per chunk
QW = 448                 # inner width: SF = 64 * 448
ROWS = SF // QW          # 64 rows -> partition p = b*64 + row
FD = UPC * QW            # free elems per component plane per chunk

F16 = np.float16
F32 = np.float32

# Set by test harness to capture an NTFF profile on the run.
TRACE = False
LAST_RESULTS = None


def _pack(d, dtype):
    """[BPC, U, SF] -> [NCH, 128, FD] with p = b*ROWS + sf//QW, f = ul*QW + sf%QW."""
    d = d.reshape(BPC, U, ROWS, QW)
    out = np.empty((NCH, BPC * ROWS, FD), dtype)
    for k in range(NCH):
        blk = d[:, k * UPC:(k + 1) * UPC]               # [BPC, UPC, ROWS, QW]
        out[k] = blk.transpose(0, 2, 1, 3).reshape(BPC * ROWS, FD)
    return out


def _unpack(t):
    """Inverse of _pack: [NCH, 128, FD] -> [BPC, U, SF]."""
    out = np.empty((BPC, U, ROWS, QW), F32)
    for k in range(NCH):
        blk = t[k].reshape(BPC, ROWS, UPC, QW).transpose(0, 2, 1, 3)
        out[:, k * UPC:(k + 1) * UPC] = blk
    return out.reshape(BPC, U, SF)


def _build_nc():
    f32 = mybir.dt.float32
    f16 = mybir.dt.float16
    nc = bass.Bass("TRN2")
    # h planes (f32): 0=m00 1=m11 2=m01 3=m10; y planes (fp16): 0=v0 1=v1
    hIn = nc.dram_tensor("hIn", [NCH, 128, 4, FD], f32, kind="ExternalInput")
    yIn = nc.dram_tensor("yIn", [NCH, 128, 2, FD], f16, kind="ExternalInput")
    xOut = nc.dram_tensor("xOut", [NCH, 128, 2, FD], f16, kind="ExternalOutput")

    with ExitStack() as ctx:
        sb = ctx.enter_context
        HF = [sb(nc.sbuf_tensor(f"HF{k}", [128, 4, FD], f32)) for k in range(NCH)]
        YB = [sb(nc.sbuf_tensor(f"YB{k}", [128, 2, FD], f16)) for k in range(NCH)]
        H6 = [sb(nc.sbuf_tensor(f"H6{k}", [128, 4, FD], f16)) for k in range(NCH)]
        P0 = [sb(nc.sbuf_tensor(f"P0{k}", [128, FD], f32)) for k in range(NCH)]
        P1 = [sb(nc.sbuf_tensor(f"P1{k}", [128, FD], f32)) for k in range(NCH)]
        DET = [sb(nc.sbuf_tensor(f"DET{k}", [128, FD], f32)) for k in range(NCH)]
        RD = [sb(nc.sbuf_tensor(f"RD{k}", [128, 1, FD], f16)) for k in range(NCH)]
        # Q plane order: 0=q0 1=q2 2=q1 3=q3 (so subR reads contiguous pairs)
        Q = [sb(nc.sbuf_tensor(f"Q{k}", [128, 4, FD], f16)) for k in range(NCH)]
        R = [sb(nc.sbuf_tensor(f"R{k}", [128, 2, FD], f16)) for k in range(NCH)]
        X = [sb(nc.sbuf_tensor(f"X{k}", [128, 2, FD], f16)) for k in range(NCH)]
        semH1 = [sb(nc.semaphore(f"semH1_{k}")) for k in range(NCH)]
        semH2 = [sb(nc.semaphore(f"semH2_{k}")) for k in range(NCH)]
        semY = [sb(nc.semaphore(f"semY{k}")) for k in range(NCH)]
        semO = [sb(nc.semaphore(f"semO{k}")) for k in range(NCH)]
        dve_sem = sb(nc.semaphore("dve_sem"))
        act_sem = sb(nc.semaphore("act_sem"))

        with nc.Block() as block:

            @block.sync
            def _(sync):
                # strict serial chain: each transfer gets full HBM bandwidth
                # and completes in consumption order
                chain = []
                for k in range(NCH):
                    chain.append((HF[k][:, 0:2, :], hIn[k][:, 0:2, :], semH1[k]))
                    chain.append((HF[k][:, 2:4, :], hIn[k][:, 2:4, :], semH2[k]))
                    chain.append((YB[k][:], yIn[k][:], semY[k]))
                prev = None
                for out, in_, sem in chain:
                    if prev is not None:
                        sync.wait_ge(prev, 16)
                    sync.dma_start(out=out, in_=in_).then_inc(sem, 16)
                    prev = sem

            # dve_sem: chunk k ops are 7k+1 .. 7k+7
            @block.vector
            def _(vector):
                for k in range(NCH):
                    c = 7 * k
                    a = 2 * k
                    vector.wait_ge(semH1[k], 16)
                    vector.tensor_mul(
                        P0[k][:], HF[k][:, 0, :], HF[k][:, 1, :]
                    ).then_inc(dve_sem, 1)                                   # c+1
                    vector.wait_ge(semH2[k], 16)
                    vector.tensor_mul(
                        P1[k][:], HF[k][:, 2, :], HF[k][:, 3, :]
                    ).then_inc(dve_sem, 1)                                   # c+2
                    vector.tensor_sub(DET[k][:], P0[k][:], P1[k][:]).then_inc(
                        dve_sem, 1
                    )                                                        # c+3
                    vector.wait_ge(act_sem, a + 1)
                    vector.wait_ge(semY[k], 16)
                    # [q0|q3] = [m11|m10] * v0   (H6 planes 1,3; out Q 0,3)
                    vector.tensor_mul(
                        Q[k][:, 0::3, :],
                        H6[k][:, 1::2, :],
                        YB[k][:, 0:1, :].broadcast_to([128, 2, FD]),
                    ).then_inc(dve_sem, 1)                                   # c+4
                    # [q2|q1] = [m00|m01] * v1   (H6 planes 0,2; out Q 1,2)
                    vector.tensor_mul(
                        Q[k][:, 1:3, :],
                        H6[k][:, 0::2, :],
                        YB[k][:, 1:2, :].broadcast_to([128, 2, FD]),
                    ).then_inc(dve_sem, 1)                                   # c+5
                    # [r0|r1] = [q0|q2] - [q1|q3]
                    vector.tensor_sub(
                        R[k][:], Q[k][:, 0:2, :], Q[k][:, 2:4, :]
                    ).then_inc(dve_sem, 1)                                   # c+6
                    vector.wait_ge(act_sem, a + 2)
                    vector.tensor_mul(
                        X[k][:], R[k][:], RD[k][:, 0:1, :].broadcast_to([128, 2, FD])
                    ).then_inc(dve_sem, 1)                                   # c+7

            @block.scalar
            def _(scalar):
                for k in range(NCH):
                    c = 7 * k
                    scalar.wait_ge(semH2[k], 16)
                    scalar.activation(
                        H6[k][:], HF[k][:], mybir.ActivationFunctionType.Copy
                    ).then_inc(act_sem, 1)                                   # a+1
                    scalar.wait_ge(dve_sem, c + 3)
                    scalar.add_instruction(
                        mybir.InstActivation(
                            name=nc.get_next_instruction_name(),
                            func=mybir.ActivationFunctionType.Reciprocal,
                            ins=[
                                scalar.lower_ap(DET[k][:]),
                                mybir.ImmediateValue(dtype=f32, value=0.0),
                                mybir.ImmediateValue(dtype=f32, value=1.0),
                                mybir.ImmediateValue(dtype=f32, value=0.0),
                            ],
                            outs=[scalar.lower_ap(RD[k][:, 0, :])],
                        )
                    ).then_inc(act_sem, 1)                                   # a+2
                for k in range(NCH):
                    scalar.wait_ge(dve_sem, 7 * k + 7)
                    scalar.dma_start(out=xOut[k], in_=X[k][:]).then_inc(semO[k], 16)
                for k in range(NCH):
                    scalar.wait_ge(semO[k], 16)

    return nc


def make_in_maps(y, h, precoding_ind):
    """Host-side gather + pack. Returns per-core input maps."""
    y = np.asarray(y)
    h = np.asarray(h)
    pi = np.asarray(precoding_ind).astype(np.int64)

    hg = h[:, pi[0]]                                     # [B, U, A, NTX, T, S, F]
    # hsel[b, u, i, j] = hg[b, u, i, 0, 2u+j]  -> components c = i*2+j
    hsel = np.stack(
        [hg[:, u, :, 0, 2 * u:2 * u + 2] for u in range(U)], axis=1
    )                                                    # [B, U, A(i), 2(j), S, F]
    hsel = np.ascontiguousarray(hsel).reshape(B, U, 4, SF).astype(F32)
    yr = np.ascontiguousarray(y).reshape(B, U, A, SF).astype(F16)

    in_maps = []
    for cid in range(NCORES):
        b0 = cid * BPC
        hs = hsel[b0:b0 + BPC]                           # [BPC, U, 4, SF]
        ys = yr[b0:b0 + BPC]                             # [BPC, U, A, SF]
        hInArr = np.empty((NCH, 128, 4, FD), F32)
        # device plane order: m00 (c0), m11 (c3), m01 (c1), m10 (c2)
        for p, c in enumerate((0, 3, 1, 2)):
            hInArr[:, :, p, :] = _pack(hs[:, :, c], F32)
        yInArr = np.empty((NCH, 128, 2, FD), F16)
        for a in range(A):
            yInArr[:, :, a, :] = _pack(ys[:, :, a], F16)
        in_maps.append({
            "hIn": np.ascontiguousarray(hInArr),
            "yIn": np.ascontiguousarray(yInArr),
        })
    return in_maps


def assemble_output(results):
    """Per-core xOut [NCH, 128, 2, FD] fp16 -> full [B, U, A, S, F] f32."""
    out = np.empty((B, U, A, S, F), F32)
    for c in range(NCORES):
        xo = np.asarray(results[c]["xOut"]).astype(F32)
        for a in range(A):
            out[c * BPC:(c + 1) * BPC, :, a] = _unpack(xo[:, :, a, :]).reshape(
                BPC, U, S, F
            )
    return out


def kernel(y, h, precoding_ind):
    global LAST_RESULTS
    in_maps = make_in_maps(y, h, precoding_ind)
    nc = _build_nc()
    res = run_bass_kernel_spmd(nc, in_maps, list(range(NCORES)), trace=TRACE)
    LAST_RESULTS = res
    return assemble_output(res.results)


# revision 10
# speedup vs baseline: 1.1696x; 1.0337x over previous
"""Block-diagonal 2x2 equalizer kernel for Trainium2 (8 NeuronCores).

Per point (b, u, s, f) solves the 2x2 system M x = v by Cramer's rule:
    m_ij = h[b, pi[u], i, 0, 2u+j, s, f]   (only 1/4 of h is needed)
    det  = m00*m11 - m01*m10
    x0   = (m11*v0 - m01*v1) / det
    x1   = (m00*v1 - m10*v0) / det
    out[b, u, a, s, f] = x_a

Sharding: data-parallel over batch, 2 batches per core on 8 cores. The host
gathers (precoding_ind) and packs operand planes into one [128, 5, 1792]
f32 HBM tensor per core: planes 0-3 = m00, m01, m11, m10 (f32 - the det
cancellation at near-singular blocks needs full input precision; fp16 h
fails at rel 0.6), plane 4 = the fp16 bits of [v0|v1] viewed as f32 words
(device reads them back via AP bitcast).  y/q/r/rdet/x run in fp16
(validated: rel err 3.3e-4 vs the 2e-2 gate) which halves their DMA bytes
and doubles DVE throughput.

DMA strategy (the v2 lesson): concurrent HWDGE DMAs round-robin at packet
granularity so ungated loads all complete together, while sem-gated serial
loads lose ~1.75us per gate to completion-receipt + re-issue latency.
Instead the input is ONE tensor sliced into 8 column strips issued on one
ring, where strip s+1 is issued after wait_ge(semL[s], 8) - half the 16
per-engine completion increments - so descriptor generation of the next
strip overlaps the tail of the previous: a gapless stream that still
completes in consumption order.  Strip widths taper up then down
(128,128,256,256,288,288,224,224) so the first compute chunk starts early
and the last loads aren't late.

Compute: 4 column chunks (256,512,576,448) of 6 DVE ops each:
    mulP   [p0|p1] = [m00|m01] * [m11|m10]      f32, 1x mode
    subDet det     = p0 - p1                     f32
    mulQA  [q0|q3] = [m11|m10] * bcast(v0)       fp16, 2x mode
    mulQB  [q2|q1] = [m00|m01] * bcast(v1)       fp16
    subR   [r0|r1] = [q0|q2] - [q1|q3]           fp16
    final  [x0|x1] = [r0|r1] * bcast(rdet)       fp16
ACT casts h f32->fp16 for the q-path (ACT is otherwise idle; the cast lets
both q muls run in DVE 2x mode), does rdet = Reciprocal(det) -> fp16 via
raw InstActivation (bass blocks the wrapper; HW spline err 2.2e-5), and
issues the 4 output stores on its own HWDGE ring.

Raw Bass (no TileContext): walrus allows one sync-wait per instruction, so
all waits are standalone wait_ge; buffers are written once per column
region (pure dataflow, no WAR hazards).
"""

from contextlib import ExitStack

import numpy as np

import concourse.bass as bass
import concourse.mybir as mybir
from concourse.bass_utils import run_bass_kernel_spmd

# Problem shapes (hardcoded per contract)
B, U, A, NTX, T, S, F = 16, 4, 2, 1, 8, 14, 2048
SF = S * F               # 28672
NCORES = 8
BPC = B // NCORES        # 2 batches per core
QW = 448                 # SF = 64 * 448
ROWS = SF // QW          # 64 rows -> partition p = b*64 + row
FW = U * QW              # 1792 full free width (u-major columns)

# column strips (DMA) and chunks (compute); strip s feeds chunk s
SW = (256, 512, 512, 512)
CW = (256, 512, 512, 512)
SOFF = [sum(SW[:i]) for i in range(len(SW) + 1)]
COFF = [sum(CW[:i]) for i in range(len(CW) + 1)]
NST = len(SW)
NCK = len(CW)

F16 = np.float16
F32 = np.float32

TRACE = False
LAST_RESULTS = None


def _pk(d, dtype):
    """[BPC, U, SF] -> [128, FW]: p = b*ROWS + sf//QW, col = u*QW + sf%QW."""
    d = d.reshape(BPC, U, ROWS, QW)
    return np.ascontiguousarray(
        d.transpose(0, 2, 1, 3).reshape(BPC * ROWS, FW)
    ).astype(dtype)


def _unpk(t):
    """Inverse of _pk: [128, FW] -> [BPC, U, SF]."""
    t = t.reshape(BPC, ROWS, U, QW).transpose(0, 2, 1, 3)
    return t.reshape(BPC, U, SF)


def _build_nc():
    f32 = mybir.dt.float32
    f16 = mybir.dt.float16
    nc = bass.Bass("TRN2")
    # planes: 0=m00 1=m01 2=m11 3=m10, 4 = fp16 [v0|v1] bits as f32 words
    hAll = nc.dram_tensor("hAll", [128, 5, FW], f32, kind="ExternalInput")
    xO = nc.dram_tensor("xO", [128, 2, FW], f16, kind="ExternalOutput")

    with ExitStack() as ctx:
        sb = ctx.enter_context
        HY = sb(nc.sbuf_tensor("HY", [128, 5, FW], f32))
        H6 = sb(nc.sbuf_tensor("H6", [128, 4, FW], f16))
        P = sb(nc.sbuf_tensor("P", [128, 2, FW], f32))
        DET = sb(nc.sbuf_tensor("DET", [128, FW], f32))
        RD = sb(nc.sbuf_tensor("RD", [128, 1, FW], f16))
        # Q plane order: 0=q0 1=q2 2=q1 3=q3
        Q = sb(nc.sbuf_tensor("Q", [128, 4, FW], f16))
        R = sb(nc.sbuf_tensor("R", [128, 2, FW], f16))
        X = sb(nc.sbuf_tensor("X", [128, 2, FW], f16))
        semL = [sb(nc.semaphore(f"semL{s}")) for s in range(NST)]
        semO = [sb(nc.semaphore(f"semO{c}")) for c in range(NCK)]
        dve_sem = sb(nc.semaphore("dve_sem"))
        act_sem = sb(nc.semaphore("act_sem"))

        yv = HY[:, 4:5, :].bitcast(f16)  # [128, 1, 2*FW]: cols 0:FW v0, FW: v1

        with nc.Block() as block:

            @block.sync
            def _(sync):
                # ungated back-to-back issue: all loads share the qSPDynamicHW
                # ring, which executes FIFO per issuing engine - strips
                # complete in order at full rate with no inter-DMA gate gaps
                for s in range(NST):
                    c0, c1 = SOFF[s], SOFF[s + 1]
                    sync.dma_start(
                        out=HY[:, :, c0:c1], in_=hAll[:, :, c0:c1]
                    ).then_inc(semL[s], 16)

            # dve_sem: chunk c ops are 6c+1 .. 6c+6
            @block.vector
            def _(vector):
                for c in range(NCK):
                    d = 6 * c
                    a = 2 * c
                    c0, c1 = COFF[c], COFF[c + 1]
                    w = c1 - c0
                    vector.wait_ge(semL[c], 16)
                    vector.tensor_mul(
                        P[:, :, c0:c1], HY[:, 0:2, c0:c1], HY[:, 2:4, c0:c1]
                    ).then_inc(dve_sem, 1)                                   # d+1
                    vector.tensor_sub(
                        DET[:, c0:c1], P[:, 0, c0:c1], P[:, 1, c0:c1]
                    ).then_inc(dve_sem, 1)                                   # d+2
                    vector.wait_ge(act_sem, a + 1)
                    # y-bits are strip-local: fp16 cols [2*c0, 2*c0+w) = v0,
                    # [2*c0+w, 2*c1) = v1 of this chunk's columns
                    vector.tensor_mul(
                        Q[:, 0::3, c0:c1],
                        H6[:, 2:4, c0:c1],
                        yv[:, :, 2 * c0:2 * c0 + w].broadcast_to([128, 2, w]),
                    ).then_inc(dve_sem, 1)                                   # d+3
                    vector.tensor_mul(
                        Q[:, 1:3, c0:c1],
                        H6[:, 0:2, c0:c1],
                        yv[:, :, 2 * c0 + w:2 * c1].broadcast_to([128, 2, w]),
                    ).then_inc(dve_sem, 1)                                   # d+4
                    vector.tensor_sub(
                        R[:, :, c0:c1], Q[:, 0:2, c0:c1], Q[:, 2:4, c0:c1]
                    ).then_inc(dve_sem, 1)                                   # d+5
                    vector.wait_ge(act_sem, a + 2)
                    vector.tensor_mul(
                        X[:, :, c0:c1],
                        R[:, :, c0:c1],
                        RD[:, :, c0:c1].broadcast_to([128, 2, w]),
                    ).then_inc(dve_sem, 1)                                   # d+6

            @block.scalar
            def _(scalar):
                f32dt = mybir.dt.float32
                for c in range(NCK):
                    d = 6 * c
                    c0, c1 = COFF[c], COFF[c + 1]
                    scalar.wait_ge(semL[c], 16)
                    scalar.activation(
                        H6[:, :, c0:c1],
                        HY[:, 0:4, c0:c1],
                        mybir.ActivationFunctionType.Copy,
                    ).then_inc(act_sem, 1)                                   # a+1
                    scalar.wait_ge(dve_sem, d + 2)
                    scalar.add_instruction(
                        mybir.InstActivation(
                            name=nc.get_next_instruction_name(),
                            func=mybir.ActivationFunctionType.Reciprocal,
                            ins=[
                                scalar.lower_ap(DET[:, c0:c1]),
                                mybir.ImmediateValue(dtype=f32dt, value=0.0),
                                mybir.ImmediateValue(dtype=f32dt, value=1.0),
                                mybir.ImmediateValue(dtype=f32dt, value=0.0),
                            ],
                            outs=[scalar.lower_ap(RD[:, 0, c0:c1])],
                        )
                    ).then_inc(act_sem, 1)                                   # a+2
                    scalar.wait_ge(dve_sem, d + 6)
                    scalar.dma_start(
                        out=xO[:, :, c0:c1], in_=X[:, :, c0:c1]
                    ).then_inc(semO[c], 16)
                for c in range(NCK):
                    scalar.wait_ge(semO[c], 16)

    return nc


def make_in_maps(y, h, precoding_ind):
    """Host-side gather + pack. Returns per-core input maps."""
    y = np.asarray(y)
    h = np.asarray(h)
    pi = np.asarray(precoding_ind).astype(np.int64)

    hg = h[:, pi[0]]                                     # [B, U, A, NTX, T, S, F]
    # hsel[b, u, i, j] = hg[b, u, i, 0, 2u+j]  -> components c = i*2+j
    hsel = np.stack(
        [hg[:, u, :, 0, 2 * u:2 * u + 2] for u in range(U)], axis=1
    )                                                    # [B, U, A(i), 2(j), S, F]
    hsel = np.ascontiguousarray(hsel).reshape(B, U, 4, SF).astype(F32)
    yr = np.ascontiguousarray(y).reshape(B, U, A, SF).astype(F16)

    in_maps = []
    for cid in range(NCORES):
        b0 = cid * BPC
        hs = hsel[b0:b0 + BPC]                           # [BPC, U, 4, SF]
        ys = yr[b0:b0 + BPC]                             # [BPC, U, A, SF]
        hA = np.empty((128, 5, FW), F32)
        # device plane order: m00 (c0), m01 (c1), m11 (c3), m10 (c2)
        for p, c in enumerate((0, 1, 3, 2)):
            hA[:, p, :] = _pk(hs[:, :, c], F32)
        v0p = _pk(ys[:, :, 0], F16)
        v1p = _pk(ys[:, :, 1], F16)                      # [128, FW] fp16 each
        # strip-local fold: plane-4 words of strip s = [v0|v1] of its columns
        for s in range(NST):
            c0, c1 = SOFF[s], SOFF[s + 1]
            yb = np.concatenate([v0p[:, c0:c1], v1p[:, c0:c1]], axis=1)
            hA[:, 4, c0:c1] = np.ascontiguousarray(yb).view(F32)
        in_maps.append({"hAll": np.ascontiguousarray(hA)})
    return in_maps


def assemble_output(results):
    """Per-core xO [128, 2, FW] fp16 -> full [B, U, A, S, F] f32."""
    out = np.empty((B, U, A, S, F), F32)
    for c in range(NCORES):
        xo = np.asarray(results[c]["xO"]).astype(F32)
        for a in range(A):
            out[c * BPC:(c + 1) * BPC, :, a] = _unpk(xo[:, a, :]).reshape(
                BPC, U, S, F
            )
    return out


def kernel(y, h, precoding_ind):
    global LAST_RESULTS
    in_maps = make_in_maps(y, h, precoding_ind)
    nc = _build_nc()
    res = run_bass_kernel_spmd(nc, in_maps, list(range(NCORES)), trace=TRACE)
    LAST_RESULTS = res
    return assemble_output(res.results)


# revision 11
# speedup vs baseline: 1.5269x; 1.3055x over previous
"""Block-diagonal 2x2 equalizer kernel for Trainium2 (8 NeuronCores).

Per point (b, u, s, f) solves the 2x2 system M x = v by Cramer's rule:
    m_ij = h[b, pi[u], i, 0, 2u+j, s, f]   (only 1/4 of h is needed)
    det  = m00*m11 - m01*m10
    x0   = (m11*v0 - m01*v1) / det
    x1   = (m00*v1 - m10*v0) / det
    out[b, u, a, s, f] = x_a

Sharding: data-parallel over batch, 2 batches per core on 8 cores. The host
gathers (precoding_ind) and packs operand planes into one [128, 5, 1792]
f32 HBM tensor per core: planes 0-3 = m00, m01, m11, m10 (f32 - the det
cancellation at near-singular blocks needs full input precision; fp16 h
fails at rel 0.6), plane 4 = the fp16 bits of [v0|v1] viewed as f32 words
(device reads them back via AP bitcast).  y/q/r/rdet/x run in fp16
(validated: rel err 3.3e-4 vs the 2e-2 gate) which halves their DMA bytes
and doubles DVE throughput.

DMA strategy (the v2 lesson): concurrent HWDGE DMAs round-robin at packet
granularity so ungated loads all complete together, while sem-gated serial
loads lose ~1.75us per gate to completion-receipt + re-issue latency.
Instead the input is ONE tensor sliced into 8 column strips issued on one
ring, where strip s+1 is issued after wait_ge(semL[s], 8) - half the 16
per-engine completion increments - so descriptor generation of the next
strip overlaps the tail of the previous: a gapless stream that still
completes in consumption order.  Strip widths taper up then down
(128,128,256,256,288,288,224,224) so the first compute chunk starts early
and the last loads aren't late.

Compute: 4 column chunks (256,512,576,448) of 6 DVE ops each:
    mulP   [p0|p1] = [m00|m01] * [m11|m10]      f32, 1x mode
    subDet det     = p0 - p1                     f32
    mulQA  [q0|q3] = [m11|m10] * bcast(v0)       fp16, 2x mode
    mulQB  [q2|q1] = [m00|m01] * bcast(v1)       fp16
    subR   [r0|r1] = [q0|q2] - [q1|q3]           fp16
    final  [x0|x1] = [r0|r1] * bcast(rdet)       fp16
ACT casts h f32->fp16 for the q-path (ACT is otherwise idle; the cast lets
both q muls run in DVE 2x mode), does rdet = Reciprocal(det) -> fp16 via
raw InstActivation (bass blocks the wrapper; HW spline err 2.2e-5), and
issues the 4 output stores on its own HWDGE ring.

Raw Bass (no TileContext): walrus allows one sync-wait per instruction, so
all waits are standalone wait_ge; buffers are written once per column
region (pure dataflow, no WAR hazards).
"""

from contextlib import ExitStack

import numpy as np

import concourse.bass as bass
import concourse.mybir as mybir
from concourse.bass_utils import run_bass_kernel_spmd

# Problem shapes (hardcoded per contract)
B, U, A, NTX, T, S, F = 16, 4, 2, 1, 8, 14, 2048
SF = S * F               # 28672
NCORES = 8
BPC = B // NCORES        # 2 batches per core
QW = 448                 # SF = 64 * 448
ROWS = SF // QW          # 64 rows -> partition p = b*64 + row
FW = U * QW              # 1792 full free width (u-major columns)

# column strips (DMA) and chunks (compute); strip s feeds chunk s
SW = (256, 512, 512, 512)
CW = (256, 512, 512, 512)
SOFF = [sum(SW[:i]) for i in range(len(SW) + 1)]
COFF = [sum(CW[:i]) for i in range(len(CW) + 1)]
NST = len(SW)
NCK = len(CW)

F16 = np.float16
F32 = np.float32

TRACE = False
LAST_RESULTS = None


def _pk(d, dtype):
    """[BPC, U, SF] -> [128, FW]: p = b*ROWS + sf//QW, col = u*QW + sf%QW."""
    d = d.reshape(BPC, U, ROWS, QW)
    return np.ascontiguousarray(
        d.transpose(0, 2, 1, 3).reshape(BPC * ROWS, FW)
    ).astype(dtype)


def _unpk(t):
    """Inverse of _pk: [128, FW] -> [BPC, U, SF]."""
    t = t.reshape(BPC, ROWS, U, QW).transpose(0, 2, 1, 3)
    return t.reshape(BPC, U, SF)


def _build_nc():
    f32 = mybir.dt.float32
    f16 = mybir.dt.float16
    nc = bass.Bass("TRN2")
    # planes: 0=m00 1=m01 2=m11 3=m10, 4 = fp16 [v0|v1] bits as f32 words
    hAll = nc.dram_tensor("hAll", [128, 5, FW], f32, kind="ExternalInput")
    xO = nc.dram_tensor("xO", [128, 2, FW], f16, kind="ExternalOutput")

    with ExitStack() as ctx:
        sb = ctx.enter_context
        HY = sb(nc.sbuf_tensor("HY", [128, 5, FW], f32))
        H6 = sb(nc.sbuf_tensor("H6", [128, 4, FW], f16))
        P = sb(nc.sbuf_tensor("P", [128, 2, FW], f32))
        DET = sb(nc.sbuf_tensor("DET", [128, FW], f32))
        RD = sb(nc.sbuf_tensor("RD", [128, 1, FW], f16))
        # Q plane order: 0=q0 1=q2 2=q1 3=q3
        Q = sb(nc.sbuf_tensor("Q", [128, 4, FW], f16))
        R = sb(nc.sbuf_tensor("R", [128, 2, FW], f16))
        X = sb(nc.sbuf_tensor("X", [128, 2, FW], f16))
        semL = [sb(nc.semaphore(f"semL{s}")) for s in range(NST)]
        semO = [sb(nc.semaphore(f"semO{c}")) for c in range(NCK)]
        dve_sem = sb(nc.semaphore("dve_sem"))
        act_sem = sb(nc.semaphore("act_sem"))

        yv = HY[:, 4:5, :].bitcast(f16)  # [128, 1, 2*FW]: cols 0:FW v0, FW: v1

        with nc.Block() as block:

            @block.sync
            def _(sync):
                # ungated back-to-back issue: all loads share the qSPDynamicHW
                # ring, which executes FIFO per issuing engine - strips
                # complete in order at full rate with no inter-DMA gate gaps
                for s in range(NST):
                    c0, c1 = SOFF[s], SOFF[s + 1]
                    sync.dma_start(
                        out=HY[:, :, c0:c1], in_=hAll[:, :, c0:c1]
                    ).then_inc(semL[s], 16)

            # dve_sem: chunk c ops are 6c+1 .. 6c+6
            @block.vector
            def _(vector):
                for c in range(NCK):
                    d = 6 * c
                    a = 2 * c
                    c0, c1 = COFF[c], COFF[c + 1]
                    w = c1 - c0
                    vector.wait_ge(semL[c], 16)
                    vector.tensor_mul(
                        P[:, :, c0:c1], HY[:, 0:2, c0:c1], HY[:, 2:4, c0:c1]
                    ).then_inc(dve_sem, 1)                                   # d+1
                    vector.tensor_sub(
                        DET[:, c0:c1], P[:, 0, c0:c1], P[:, 1, c0:c1]
                    ).then_inc(dve_sem, 1)                                   # d+2
                    vector.wait_ge(act_sem, a + 1)
                    # y-bits are strip-local: fp16 cols [2*c0, 2*c0+w) = v0,
                    # [2*c0+w, 2*c1) = v1 of this chunk's columns
                    vector.tensor_mul(
                        Q[:, 0::3, c0:c1],
                        H6[:, 2:4, c0:c1],
                        yv[:, :, 2 * c0:2 * c0 + w].broadcast_to([128, 2, w]),
                    ).then_inc(dve_sem, 1)                                   # d+3
                    vector.tensor_mul(
                        Q[:, 1:3, c0:c1],
                        H6[:, 0:2, c0:c1],
                        yv[:, :, 2 * c0 + w:2 * c1].broadcast_to([128, 2, w]),
                    ).then_inc(dve_sem, 1)                                   # d+4
                    vector.tensor_sub(
                        R[:, :, c0:c1], Q[:, 0:2, c0:c1], Q[:, 2:4, c0:c1]
                    ).then_inc(dve_sem, 1)                                   # d+5
                    vector.wait_ge(act_sem, a + 2)
                    vector.tensor_mul(
                        X[:, :, c0:c1],
                        R[:, :, c0:c1],
                        RD[:, :, c0:c1].broadcast_to([128, 2, w]),
                    ).then_inc(dve_sem, 1)                                   # d+6

            @block.scalar
            def _(scalar):
                f32dt = mybir.dt.float32

                def cast(c):
                    c0, c1 = COFF[c], COFF[c + 1]
                    scalar.wait_ge(semL[c], 16)
                    scalar.activation(
                        H6[:, :, c0:c1],
                        HY[:, 0:4, c0:c1],
                        mybir.ActivationFunctionType.Copy,
                    ).then_inc(act_sem, 1)

                def recip(c):
                    c0, c1 = COFF[c], COFF[c + 1]
                    scalar.wait_ge(dve_sem, 6 * c + 2)
                    scalar.add_instruction(
                        mybir.InstActivation(
                            name=nc.get_next_instruction_name(),
                            func=mybir.ActivationFunctionType.Reciprocal,
                            ins=[
                                scalar.lower_ap(DET[:, c0:c1]),
                                mybir.ImmediateValue(dtype=f32dt, value=0.0),
                                mybir.ImmediateValue(dtype=f32dt, value=1.0),
                                mybir.ImmediateValue(dtype=f32dt, value=0.0),
                            ],
                            outs=[scalar.lower_ap(RD[:, 0, c0:c1])],
                        )
                    ).then_inc(act_sem, 1)

                def store(c):
                    c0, c1 = COFF[c], COFF[c + 1]
                    scalar.wait_ge(dve_sem, 6 * c + 6)
                    scalar.dma_start(
                        out=xO[:, :, c0:c1], in_=X[:, :, c0:c1]
                    ).then_inc(semO[c], 16)

                # order: each cast as soon as its strip lands, never stuck
                # behind a store (stores wait on DVE finals). act_sem counts
                # must stay (cast c, recip c) = (2c+1, 2c+2) for DVE's waits,
                # so emit cast/recip in chunk order, stores as late as allowed.
                cast(0); recip(0); cast(1); store(0)
                recip(1); cast(2); store(1)
                recip(2); cast(3); store(2)
                recip(3); store(3)
                for c in range(NCK):
                    scalar.wait_ge(semO[c], 16)

    return nc


def make_in_maps(y, h, precoding_ind):
    """Host-side gather + pack. Returns per-core input maps."""
    y = np.asarray(y)
    h = np.asarray(h)
    pi = np.asarray(precoding_ind).astype(np.int64)

    hg = h[:, pi[0]]                                     # [B, U, A, NTX, T, S, F]
    # hsel[b, u, i, j] = hg[b, u, i, 0, 2u+j]  -> components c = i*2+j
    hsel = np.stack(
        [hg[:, u, :, 0, 2 * u:2 * u + 2] for u in range(U)], axis=1
    )                                                    # [B, U, A(i), 2(j), S, F]
    hsel = np.ascontiguousarray(hsel).reshape(B, U, 4, SF).astype(F32)
    yr = np.ascontiguousarray(y).reshape(B, U, A, SF).astype(F16)

    in_maps = []
    for cid in range(NCORES):
        b0 = cid * BPC
        hs = hsel[b0:b0 + BPC]                           # [BPC, U, 4, SF]
        ys = yr[b0:b0 + BPC]                             # [BPC, U, A, SF]
        hA = np.empty((128, 5, FW), F32)
        # device plane order: m00 (c0), m01 (c1), m11 (c3), m10 (c2)
        for p, c in enumerate((0, 1, 3, 2)):
            hA[:, p, :] = _pk(hs[:, :, c], F32)
        v0p = _pk(ys[:, :, 0], F16)
        v1p = _pk(ys[:, :, 1], F16)                      # [128, FW] fp16 each
        # strip-local fold: plane-4 words of strip s = [v0|v1] of its columns
        for s in range(NST):
            c0, c1 = SOFF[s], SOFF[s + 1]
            yb = np.concatenate([v0p[:, c0:c1], v1p[:, c0:c1]], axis=1)
            hA[:, 4, c0:c1] = np.ascontiguousarray(yb).view(F32)
        in_maps.append({"hAll": np.ascontiguousarray(hA)})
    return in_maps


def assemble_output(results):
    """Per-core xO [128, 2, FW] fp16 -> full [B, U, A, S, F] f32."""
    out = np.empty((B, U, A, S, F), F32)
    for c in range(NCORES):
        xo = np.asarray(results[c]["xO"]).astype(F32)
        for a in range(A):
            out[c * BPC:(c + 1) * BPC, :, a] = _unpk(xo[:, a, :]).reshape(
                BPC, U, S, F
            )
    return out


def kernel(y, h, precoding_ind):
    global LAST_RESULTS
    in_maps = make_in_maps(y, h, precoding_ind)
    nc = _build_nc()
    res = run_bass_kernel_spmd(nc, in_maps, list(range(NCORES)), trace=TRACE)
    LAST_RESULTS = res
    return assemble_output(res.results)
